# revision 10
# baseline (speedup 1.0000x reference)
"""Trainium2 Bass kernel for nn_MessagePassing (GNN last-writer message passing).

Math (from the reference):
  src[l,j]  = max{ i : adj[l,i,j]==1 } U {j}          (last writer wins)
  deg[l,i]  = 1 + sum_j adj[l,i,j]
  out[j,l,:] = (W @ feature[src[l,j], l, :]) / sqrt(deg[l,src]*deg[l,j])

Strategy (8 NeuronCores, one chip):
  - Shard destinations j in contiguous blocks of 512 per core.
  - Host packs adj[l, :, j_shard] transposed to [L, JJ, N] uint8 (0 or 0x38).
    0x38 doubles as fp8_e4m3 1.0 so the TensorEngine can sum it for degrees.
  - src via hierarchical max: int32-reinterpret the uint8 stream (4 lanes/elem),
    segmented reduce_max finds the last 128-wide i-block with any edge, then a
    small indirect-DMA gather re-reads only the winning 128B blocks to find the
    exact source index.
  - Per-core partial degrees (PE matmul with ones) are AllReduced (128KB).
  - Features are gathered by src via indirect DMA (2MB instead of streaming
    16MB), transposed on PE, multiplied by W^T, scaled, and written out.
"""

import os
import sys
import types
from contextlib import ExitStack

import numpy as np

for _p in ("/opt/trn_rl_repo",):
    if os.path.isdir(_p) and _p not in sys.path:
        sys.path.append(_p)

from concourse import bacc, bass, mybir
from concourse.masks import make_identity
from concourse.tile import TileContext

F32 = mybir.dt.float32
I32 = mybir.dt.int32
U8 = mybir.dt.uint8
FP8 = mybir.dt.float8e4
AX = mybir.AxisListType
OP = mybir.AluOpType
ACT = mybir.ActivationFunctionType

P = 128  # SBUF partitions
EDGE = 0x38  # adjacency byte: fp8_e4m3 bit pattern of 1.0

N_NODES, N_LAYERS, D, N_CORES = 4096, 8, 128, 8


def _install_ntff_hook():
    """This image's antenv lacks axon_hooks; synthesize it so trace=True works."""
    try:
        import antenv
    except ImportError:
        return
    if "antenv.axon_hooks" in sys.modules:
        return
    mod = types.ModuleType("antenv.axon_hooks")
    _state = {"hook": None}
    mod.set_axon_ntff_profile_hook = lambda h: _state.__setitem__("hook", h)
    mod.get_axon_ntff_profile_hook = lambda: _state["hook"]
    sys.modules["antenv.axon_hooks"] = mod
    antenv.axon_hooks = mod
    try:
        from trn_agent_boot.trn_boot import _ntff_profile_via_ctypes

        mod.set_axon_ntff_profile_hook(
            _ntff_profile_via_ctypes("/opt/axon/libaxon_pjrt.so")
        )
    except Exception:
        pass


def build_kernel(N=N_NODES, L=N_LAYERS, JJ=None, n_cores=N_CORES):
    """One SPMD program, identical on all cores; per-core data arrives via inputs."""
    if JJ is None:
        JJ = N // n_cores
    T = JJ // P  # 128-row destination tiles per layer
    G = L * T  # total (layer, tile) groups
    NBLK = N // P  # 128-wide source blocks per adjacency row
    IB = P // 4  # int32 words per source block

    nc = bacc.Bacc()
    padjT = nc.declare_dram_parameter("padjT", [L, JJ, N], U8, isOutput=False)
    featg = nc.declare_dram_parameter("featg", [N * L, D], F32, isOutput=False)
    wt = nc.declare_dram_parameter("wt", [D, D], F32, isOutput=False)
    jg = nc.declare_dram_parameter("jg", [P, G], F32, isOutput=False)
    # host-precomputed index constants (device iota is a GPSIMD software op
    # whose multi-dim/zero-step patterns differ between sim and HW)
    iotb = nc.declare_dram_parameter("iotb", [P, G * NBLK], F32, isOutput=False)
    iotr = nc.declare_dram_parameter("iotr", [P, G * P], F32, isOutput=False)
    cblk = nc.declare_dram_parameter("cblk", [P, G], F32, isOutput=False)
    cladd = nc.declare_dram_parameter("cladd", [P, G], F32, isOutput=False)
    clN = nc.declare_dram_parameter("clN", [P, G], F32, isOutput=False)
    out = nc.declare_dram_parameter("out", [JJ, L, D], F32, isOutput=True)

    deg_in = nc.dram_tensor("deg_in", [L, N], F32)
    deg_out = nc.dram_tensor("deg_out", [L * N, 1], F32, addr_space="Shared")

    with TileContext(nc) as tc, ExitStack() as ctx:
        const = ctx.enter_context(tc.tile_pool(name="const", bufs=1))
        adjp = ctx.enter_context(tc.tile_pool(name="adjp", bufs=3))
        keep = ctx.enter_context(tc.tile_pool(name="keep", bufs=1))
        wrk = ctx.enter_context(tc.tile_pool(name="wrk", bufs=1))
        sml = ctx.enter_context(tc.tile_pool(name="sml", bufs=1))
        mm = ctx.enter_context(tc.tile_pool(name="mm", bufs=3))
        psum = ctx.enter_context(tc.tile_pool(name="psum", bufs=2, space="PSUM"))
        psdeg = ctx.enter_context(tc.tile_pool(name="psdeg", bufs=2, space="PSUM"))

        # ---- constants ----
        eye = const.tile([P, P], F32)
        make_identity(nc, eye[:])
        wt_sb = const.tile([D, D], F32)
        nc.sync.dma_start(wt_sb[:], wt[:])
        jg_sb = const.tile([P, G], F32)
        nc.sync.dma_start(jg_sb[:], jg[:])
        ones8 = const.tile([P, 1], U8)
        nc.vector.memset(ones8[:], EDGE)

        def load_const(name, param, shape):
            t = const.tile(shape, F32, tag=name)
            nc.sync.dma_start(t[:], param.ap())
            return t

        iotaB1 = load_const("iotaB1", iotb, [P, G * NBLK])  # b+1 tiled per group
        iotaR1 = load_const("iotaR1", iotr, [P, G * P])  # r+1 tiled per group
        blkbase = load_const("blkbase", cblk, [P, G])  # (l*JJ+t*128+p)*NBLK
        ladd = load_const("ladd", cladd, [P, G])  # l per group
        lN = load_const("lN", clN, [P, G])  # l*N per group

        # ---- phase 1: stream adjacency; per-block any-edge + degree partials ----
        anyb = keep.tile([P, G * NBLK], I32)
        for l in range(L):
            dacc = sml.tile([P, NBLK], F32, tag="dacc")
            for t in range(T):
                g = l * T + t
                at = adjp.tile([P, N], U8, tag="adj")
                nc.sync.dma_start(at[:], padjT[l, t * P : (t + 1) * P, :])
                nc.vector.reduce_max(
                    anyb[:, g * NBLK : (g + 1) * NBLK],
                    at[:].bitcast(I32).rearrange("p (b w) -> p b w", w=IB),
                    axis=AX.X,
                )
                af8 = at[:].bitcast(FP8)
                degp = psdeg.tile([P, NBLK], F32, tag="degp")
                for w in range(NBLK):
                    nc.tensor.matmul(
                        degp[:, w : w + 1],
                        lhsT=af8[:, w * P : (w + 1) * P],
                        rhs=ones8[:].bitcast(FP8),
                        start=True,
                        stop=True,
                    )
                if t == 0:
                    nc.scalar.copy(dacc[:], degp[:])
                else:
                    nc.vector.tensor_tensor(dacc[:], dacc[:], degp[:], op=OP.add)
            nc.sync.dma_start(deg_in[l, :].rearrange("(w p) -> p w", p=P), dacc[:])

        # ---- cross-core degree AllReduce (128KB; overlaps with phase 2) ----
        nc.gpsimd.collective_compute(
            "AllReduce",
            OP.add,
            ins=[deg_in.ap().opt()],
            outs=[deg_out.ap().opt()],
            replica_groups=[list(range(n_cores))],
        )

        # ---- phase 2: block argmax -> refine gather -> exact src ----
        nzb = wrk.tile([P, G * NBLK], F32)
        nc.vector.tensor_scalar(nzb[:], anyb[:], 0, None, OP.is_gt)
        bsel = wrk.tile([P, G * NBLK], F32)
        nc.vector.tensor_tensor(bsel[:], nzb[:], iotaB1[:], op=OP.mult)
        Bp1 = sml.tile([P, G], F32)
        nc.vector.reduce_max(
            Bp1[:], bsel[:].rearrange("p (g b) -> p g b", b=NBLK), axis=AX.X
        )
        B = sml.tile([P, G], F32)
        nc.vector.tensor_scalar(B[:], Bp1[:], 1, None, OP.subtract)  # -1 if none
        Brelu = sml.tile([P, G], F32)
        nc.vector.tensor_scalar(Brelu[:], B[:], 0, None, OP.max)
        ridxf = sml.tile([P, G], F32)
        nc.vector.tensor_tensor(ridxf[:], Brelu[:], blkbase[:], op=OP.add)
        ridxi = sml.tile([P, G], I32)
        nc.vector.tensor_copy(ridxi[:], ridxf[:])

        # NOTE: HW indirect DMA honors only ONE offset per partition per call
        # (multi-column offset tiles silently gather consecutive rows), so
        # every gather below is a [P, 1]-offset call.
        refblk = wrk.tile([P, G * P], U8)
        padj_blocks = padjT.ap().rearrange("l j (b k) -> (l j b) k", k=P)
        for g in range(G):
            nc.gpsimd.indirect_dma_start(
                out=refblk[:, g * P : (g + 1) * P],
                out_offset=None,
                in_=padj_blocks,
                in_offset=bass.IndirectOffsetOnAxis(ap=ridxi[:, g : g + 1], axis=0),
            )
        rsel = wrk.tile([P, G * P], F32)
        nc.vector.tensor_tensor(rsel[:], refblk[:], iotaR1[:], op=OP.mult)
        Rp1 = sml.tile([P, G], F32)  # 0x38*(r+1), or 0
        nc.vector.reduce_max(
            Rp1[:], rsel[:].rearrange("p (g r) -> p g r", r=P), axis=AX.X
        )
        srcadj = sml.tile([P, G], F32)
        nc.vector.tensor_scalar(srcadj[:], B[:], float(P), -1.0, OP.mult, OP.add)
        rplus = sml.tile([P, G], F32)
        nc.vector.tensor_scalar(rplus[:], Rp1[:], 1.0 / EDGE, None, OP.mult)
        nc.vector.tensor_tensor(srcadj[:], srcadj[:], rplus[:], op=OP.add)
        src = sml.tile([P, G], F32)
        nc.vector.tensor_tensor(src[:], srcadj[:], jg_sb[:], op=OP.max)

        # gather indices: feature row = src*L + l ; degree rows = l*N + {src, j}
        fidxf = sml.tile([P, G], F32)
        nc.vector.tensor_scalar(fidxf[:], src[:], float(L), None, OP.mult)
        nc.vector.tensor_tensor(fidxf[:], fidxf[:], ladd[:], op=OP.add)
        fidxi = sml.tile([P, G], I32)
        nc.vector.tensor_copy(fidxi[:], fidxf[:])
        didxf = sml.tile([P, 2 * G], F32)
        nc.vector.tensor_tensor(didxf[:, 0:G], src[:], lN[:], op=OP.add)
        nc.vector.tensor_tensor(didxf[:, G : 2 * G], jg_sb[:], lN[:], op=OP.add)
        didxi = sml.tile([P, 2 * G], I32)
        nc.vector.tensor_copy(didxi[:], didxf[:])

        degv = sml.tile([P, 2 * G], F32)
        for g in range(2 * G):
            nc.gpsimd.indirect_dma_start(
                out=degv[:, g : g + 1],
                out_offset=None,
                in_=deg_out.ap(),
                in_offset=bass.IndirectOffsetOnAxis(ap=didxi[:, g : g + 1], axis=0),
            )
        deg1 = sml.tile([P, 2 * G], F32)
        nc.vector.tensor_scalar(deg1[:], degv[:], 1.0, None, OP.add)  # + self loop
        prod = sml.tile([P, G], F32)
        nc.vector.tensor_tensor(prod[:], deg1[:, 0:G], deg1[:, G : 2 * G], op=OP.mult)
        sq = sml.tile([P, G], F32)
        nc.scalar.activation(sq[:], prod[:], ACT.Sqrt)
        scale = sml.tile([P, G], F32)
        nc.vector.reciprocal(scale[:], sq[:])

        featsb = keep.tile([P, G * D], F32)
        for g in range(G):
            nc.gpsimd.indirect_dma_start(
                out=featsb[:, g * D : (g + 1) * D],
                out_offset=None,
                in_=featg.ap(),
                in_offset=bass.IndirectOffsetOnAxis(ap=fidxi[:, g : g + 1], axis=0),
            )

        # ---- phase 3: W @ gathered features, scale, write out ----
        for g in range(G):
            l, t = divmod(g, T)
            pt = psum.tile([P, P], F32, tag="pt")
            nc.tensor.transpose(pt[:], featsb[:, g * D : (g + 1) * D], eye[:])
            gt = mm.tile([P, P], F32, tag="gt")
            nc.scalar.copy(gt[:], pt[:])
            po = psum.tile([P, P], F32, tag="po")
            nc.tensor.matmul(po[:], lhsT=gt[:], rhs=wt_sb[:], start=True, stop=True)
            osb = mm.tile([P, P], F32, tag="osb")
            nc.vector.tensor_scalar(osb[:], po[:], scale[:, g : g + 1], None, OP.mult)
            nc.sync.dma_start(out[t * P : (t + 1) * P, l, :], osb[:])

    nc.finalize()
    return nc


def shard_inputs(feature, W, adj, N=N_NODES, L=N_LAYERS, n_cores=N_CORES):
    """Host-side sharding/layout prep. Elementwise transforms only."""
    JJ = N // n_cores
    T = JJ // P
    G = L * T
    NBLK = N // P
    featg = np.ascontiguousarray(feature.reshape(N * L, D).astype(np.float32))
    wt = np.ascontiguousarray(np.asarray(W, dtype=np.float32).T)
    iotb = np.tile(np.arange(1, NBLK + 1, dtype=np.float32), (P, G)).reshape(
        P, G * NBLK
    )
    iotr = np.tile(np.arange(1, P + 1, dtype=np.float32), (P, G)).reshape(P, G * P)
    gl = np.repeat(np.arange(L), T).astype(np.float32)  # l per group
    gt = np.tile(np.arange(T), L).astype(np.float32)  # t per group
    pp = np.arange(P, dtype=np.float32)[:, None]
    cblk = (gl[None, :] * JJ + gt[None, :] * P + pp) * NBLK
    cladd = np.tile(gl, (P, 1))
    clN = cladd * N
    common = {
        "featg": featg,
        "wt": wt,
        "iotb": iotb,
        "iotr": iotr,
        "cblk": cblk.astype(np.float32),
        "cladd": cladd.astype(np.float32),
        "clN": clN.astype(np.float32),
    }
    in_maps = []
    for c in range(n_cores):
        sl = adj[:, :, c * JJ : (c + 1) * JJ]  # [L, N, JJ]
        padjT = np.ascontiguousarray(
            (sl.transpose(0, 2, 1) == 1).astype(np.uint8) * np.uint8(EDGE)
        )
        jgv = (c * JJ + gt[None, :] * P + pp).astype(np.float32)
        in_maps.append({"padjT": padjT, "jg": jgv, **common})
    return in_maps


_NC_CACHE = {}
LAST_RESULT = None


def kernel(feature, W, adj):
    global LAST_RESULT
    _install_ntff_hook()
    from concourse.bass_utils import run_bass_kernel_spmd

    feature = np.asarray(feature)
    W = np.asarray(W)
    adj = np.asarray(adj)
    N, L, _ = feature.shape
    key = (N, L)
    if key not in _NC_CACHE:
        _NC_CACHE[key] = build_kernel(N=N, L=L)
    nc = _NC_CACHE[key]

    in_maps = shard_inputs(feature, W, adj, N=N, L=L)
    res = run_bass_kernel_spmd(nc, in_maps, core_ids=list(range(N_CORES)))
    LAST_RESULT = res
    return np.concatenate([res.results[c]["out"] for c in range(N_CORES)], axis=0)
